# revision 1
# baseline (speedup 1.0000x reference)
"""Cross-attention Trainium2 kernel, 8-way head-sharded (tensor parallel).

Strategy (per spec sharding_hint): split the 16 heads across the 8 cores
(2 heads / core) by slicing Wq/Wk/Wv column-wise (rows of the [out,in]
weight) and Wo row-wise. Each core computes q/k/v projections for its
128-dim slice, the masked-softmax attention for its 2 heads, then the
full-C output projection for a 1/8 slice of the (B*N) rows after an
AllToAll that redistributes the per-core attention outputs from
head-sharded to row-sharded. Host concatenates the 8 row slices.

All matmuls run in float32r (TF32-like: 1s/8e/11m) at full PE rate with
fp32 PSUM accumulation; end-to-end relative error ~1e-4.

Softmax is computed without max-subtraction (logits are O(3) for this
problem's distributions) as exp(S)*mask / sum(exp(S)*mask); the mask
enters through a masked V and an extra mask column appended to V that
yields the denominator inside the same PE accumulation as E@V.
"""
import sys
sys.path.insert(0, '/opt/trn_rl_repo')

import numpy as np

B, N, M, C, H, D = 4, 512, 2048, 1024, 16, 64
R = 8               # cores
DL = C // R         # per-core q/k/v slice width (2 heads x 64)
SCALE = D ** -0.5
BN, BM = B * N, B * M
CC = C // 128       # contraction chunks
MT = M // 128       # m-tiles per batch
P = 128

_cached = {}


def to_f32r(a):
    """Round fp32 -> float32r (1s/8e/11m in high 20 bits), RNE. Bit-exact
    with the hardware's cast (verified on device)."""
    a = np.ascontiguousarray(a, dtype=np.float32)
    u = a.view(np.uint32)
    keep = u & np.uint32(0xFFFFF000)
    rem = u & np.uint32(0xFFF)
    half = np.uint32(0x800)
    lsb = (keep >> np.uint32(12)) & np.uint32(1)
    round_up = (rem > half) | ((rem == half) & (lsb == 1))
    return (keep + np.where(round_up, np.uint32(0x1000), np.uint32(0))).view(np.float32)


def _build():
    import concourse.tile as tile
    from concourse import bacc, mybir
    from concourse.masks import make_identity
    from contextlib import ExitStack

    F32 = mybir.dt.float32
    F32R = mybir.dt.float32r
    I32 = mybir.dt.int32
    AF = mybir.ActivationFunctionType
    OP = mybir.AluOpType

    nc = bacc.Bacc("TRN2", target_bir_lowering=False, debug=False, num_devices=R)

    xin = nc.dram_tensor("xin", [BN, C], F32, kind="ExternalInput").ap()
    ctx = nc.dram_tensor("ctx", [BM, C], F32, kind="ExternalInput").ap()
    mask_d = nc.dram_tensor("mask", [B, M], I32, kind="ExternalInput").ap()
    wq_d = nc.dram_tensor("wq", [C, DL], F32R, kind="ExternalInput").ap()
    wk_d = nc.dram_tensor("wk", [C, DL], F32R, kind="ExternalInput").ap()
    wv_d = nc.dram_tensor("wv", [C, DL], F32R, kind="ExternalInput").ap()
    wo_d = nc.dram_tensor("wo", [C, C], F32R, kind="ExternalInput").ap()
    bq_d = nc.dram_tensor("bq", [DL, 1], F32, kind="ExternalInput").ap()
    bk_d = nc.dram_tensor("bk", [DL, 1], F32, kind="ExternalInput").ap()
    bo_d = nc.dram_tensor("bo", [P, C], F32, kind="ExternalInput").ap()
    out_d = nc.dram_tensor("out", [2, P, C], F32, kind="ExternalOutput").ap()

    with tile.TileContext(nc) as tc, ExitStack() as es:
        const = es.enter_context(tc.tile_pool(name="const", bufs=1))
        kt_pool = es.enter_context(tc.tile_pool(name="kt", bufs=8))
        vn_pool = es.enter_context(tc.tile_pool(name="vn", bufs=32))
        qt_pool = es.enter_context(tc.tile_pool(name="qt", bufs=4))
        av_pool = es.enter_context(tc.tile_pool(name="av", bufs=2))
        agp = es.enter_context(tc.tile_pool(name="agp", bufs=2))
        outp = es.enter_context(tc.tile_pool(name="outp", bufs=4))
        dram = es.enter_context(tc.tile_pool(name="dram", bufs=1, space="DRAM"))
        pst = es.enter_context(tc.tile_pool(name="pst", bufs=2, space="PSUM"))
        psp = es.enter_context(tc.tile_pool(name="psp", bufs=2, space="PSUM"))
        pss = es.enter_context(tc.tile_pool(name="pss", bufs=2, space="PSUM"))
        psa = es.enter_context(tc.tile_pool(name="psa", bufs=2, space="PSUM"))

        # ---- constants ----
        wq_t = const.tile([P, CC, DL], F32R, tag="wq")
        nc.sync.dma_start(wq_t[:], wq_d.rearrange("(cc p) d -> p cc d", p=P))
        wk_t = const.tile([P, CC, DL], F32R, tag="wk")
        nc.sync.dma_start(wk_t[:], wk_d.rearrange("(cc p) d -> p cc d", p=P))
        wv_t = const.tile([P, CC, DL], F32R, tag="wv")
        nc.sync.dma_start(wv_t[:], wv_d.rearrange("(cc p) d -> p cc d", p=P))
        bq_t = const.tile([P, 1], F32, tag="bq")
        nc.sync.dma_start(bq_t[:], bq_d[:])
        bk_t = const.tile([P, 1], F32, tag="bk")
        nc.sync.dma_start(bk_t[:], bk_d[:])
        bo_t = const.tile([P, C], F32, tag="bo")
        nc.sync.dma_start(bo_t[:], bo_d[:])
        mi = const.tile([P, B, MT], I32, tag="mi")
        nc.sync.dma_start(mi[:], mask_d.rearrange("b (mt p) -> p b mt", p=P))
        mf = const.tile([P, B, MT], F32, tag="mf")
        nc.vector.tensor_copy(mf[:], mi[:])
        identf = const.tile([P, P], F32, tag="idf")
        make_identity(nc, identf[:])
        identr = const.tile([P, P], F32R, tag="idr")
        nc.vector.tensor_copy(identr[:], identf[:])
        ones_f = const.tile([1, 64], F32, tag="onesf")
        nc.gpsimd.memset(ones_f[:], 1.0)
        ones_r = const.tile([1, 64], F32R, tag="ones")
        nc.vector.tensor_copy(ones_r[:], ones_f[:])

        es2 = ExitStack()
        xn_pool = es2.enter_context(tc.tile_pool(name="xn", bufs=2))
        tT_pool = es2.enter_context(tc.tile_pool(name="tT", bufs=2))
        vt_pool = es2.enter_context(tc.tile_pool(name="vt", bufs=2))
        e_pool = es2.enter_context(tc.tile_pool(name="e", bufs=6))
        nrm_pool = es2.enter_context(tc.tile_pool(name="nrm", bufs=2))

        kt_tiles = []
        vn_tiles = []
        qt_tiles = []
        av_tiles = []
        a2a_outs = []

        def transpose_slab(slab):
            """[128, 4, 1024] fp32 natural rows -> [128, CC, 512] f32r transposed."""
            tt = tT_pool.tile([P, CC, 512], F32R, tag="tt")
            for cc in range(CC):
                pt = pst.tile([P, 512], F32, tag="t")
                for j in range(4):
                    nc.tensor.transpose(pt[:, j * P:(j + 1) * P],
                                        slab[:, j, cc * P:(cc + 1) * P],
                                        identf[:])
                nc.vector.tensor_copy(tt[:, cc, :], pt[:])
            return tt

        # ---- Phase A: x -> qT per batch ----
        for b in range(B):
            xslab = xn_pool.tile([P, 4, C], F32, tag="slab")
            nc.sync.dma_start(
                xslab[:],
                xin[b * N:(b + 1) * N, :].rearrange("(j p) c -> p j c", p=P))
            xt = transpose_slab(xslab)
            pq = psp.tile([P, 512], F32, tag="p")
            for cc in range(CC):
                nc.tensor.matmul(pq[:], lhsT=wq_t[:, cc, :], rhs=xt[:, cc, :],
                                 start=(cc == 0), stop=(cc == CC - 1))
            qt = qt_pool.tile([P, 512], F32R, tag="qt")
            nc.scalar.activation(qt[:], pq[:], AF.Identity, bias=bq_t[:], scale=1.0)
            qt_tiles.append(qt)

        # ---- Phase C body (emitted interleaved with Phase B) ----
        def attention_batch(b):
            pav = [psa.tile([P, 512], F32, tag="a", name=f"pav{_h}") for _h in range(2)]
            for mt in range(MT):
                tm = b * MT + mt
                kc, off = tm // 4, (tm % 4) * P
                kt = kt_tiles[kc]
                vt_t = vn_tiles[tm]
                ps = [pss.tile([P, 512], F32, tag="s", name=f"ps{_h}") for _h in range(2)]
                for h in range(2):
                    nc.tensor.matmul(ps[h][:],
                                     lhsT=kt[h * 64:(h + 1) * 64, off:off + P],
                                     rhs=qt_tiles[b][h * 64:(h + 1) * 64, :],
                                     start=True, stop=True)
                ee = []
                for h in range(2):
                    e = e_pool.tile([P, 512], F32R, tag="e")
                    nc.scalar.activation(e[:], ps[h][:], AF.Exp,
                                         bias=0.0, scale=float(SCALE))
                    ee.append(e)
                first, last = (mt == 0), (mt == MT - 1)
                for h in range(2):
                    # lhsT = [V_h | mask]: rows 0:64 = EV^T, row 64 = denominator
                    nc.tensor.matmul(pav[h][0:65, :], lhsT=vt_t[:, h, :],
                                     rhs=ee[h][:], start=first, stop=last)
            avt = av_pool.tile([P, 512], F32R, tag="av")
            for h in range(2):
                rec = nrm_pool.tile([1, 512], F32, tag="rec")
                nc.vector.reciprocal(rec[:], pav[h][64:65, :])
                rec_r = nrm_pool.tile([1, 512], F32R, tag="recr")
                nc.vector.tensor_copy(rec_r[:], rec[:])
                pb = psp.tile([P, 512], F32, tag="p")
                nc.tensor.matmul(pb[0:64, :], lhsT=ones_r[:], rhs=rec_r[:],
                                 start=True, stop=True)
                bc = nrm_pool.tile([64, 512], F32, tag="bc")
                nc.scalar.copy(bc[:], pb[0:64, :])
                nc.vector.scalar_tensor_tensor(
                    out=avt[h * 64:(h + 1) * 64, :], in0=pav[h][0:64, :],
                    scalar=1.0, in1=bc[:], op0=OP.mult, op1=OP.mult)
            av_tiles.append(avt)
            if b % 2 == 1:
                h = b // 2
                a2a_in = dram.tile([R, P, P], F32R, name=f"a2ai{h}")
                a2a_out = dram.tile([R, P, P], F32R, name=f"a2ao{h}")
                for j in range(R):
                    src = av_tiles[2 * h + j // 4]
                    nc.scalar.dma_start(a2a_in[j, :, :],
                                        src[:, (j % 4) * P:(j % 4 + 1) * P])
                nc.gpsimd.collective_compute(
                    "AllToAll", OP.bypass, replica_groups=[list(range(R))],
                    ins=[a2a_in.opt()], outs=[a2a_out.opt()])
                a2a_outs.append(a2a_out)

        def wo_half(h):
            """Deferred behind the a2a so the collective completes off the
            critical path (in-order engine queues)."""
            agt = agp.tile([P, CC, P], F32R, name=f"agt{h}", tag="ag")
            nc.scalar.dma_start(agt[:], a2a_outs[h].rearrange("i p n -> p i n"))
            for ch in range(2):
                po = psp.tile([P, 512], F32, tag="p", name=f"po{h}_{ch}")
                for cc in range(CC):
                    nc.tensor.matmul(po[:], lhsT=agt[:, cc, :],
                                     rhs=wo_t[:, cc, ch * 512:(ch + 1) * 512],
                                     start=(cc == 0), stop=(cc == CC - 1))
                ob = outp.tile([P, 512], F32, tag="ob", name=f"ob{h}_{ch}")
                nc.vector.scalar_tensor_tensor(
                    out=ob[:], in0=po[:], scalar=1.0,
                    in1=bo_t[:, ch * 512:(ch + 1) * 512],
                    op0=OP.mult, op1=OP.add)
                nc.scalar.dma_start(out_d[h, :, ch * 512:(ch + 1) * 512], ob[:])

        # tiny collective up front: absorbs cross-core start skew off the
        # critical path so the first real AllToAll doesn't eat it.
        bar_in = dram.tile([1, 4], F32, tag="barin")
        bar_out = dram.tile([R, 4], F32, tag="barout")
        nc.sync.dma_start(bar_in[:], bq_d.rearrange("d o -> o d")[0:1, 0:4])
        nc.gpsimd.collective_compute(
            "AllGather", OP.bypass, replica_groups=[list(range(R))],
            ins=[bar_in.opt()], outs=[bar_out.opt()])

        # output projection weights: needed first at end of C(0); emit the
        # DMA here so it overlaps the ctx pipeline.
        wo_t = const.tile([P, CC, C], F32R, tag="wo")
        nc.sync.dma_start(wo_t[:], wo_d.rearrange("(cc p) c -> p cc c", p=P))

        # ---- Phase B: ctx -> kT, V_aug (+ attention per finished batch) ----
        for mc in range(BM // 512):
            cslab = xn_pool.tile([P, 4, C], F32, tag="slab")
            nc.sync.dma_start(
                cslab[:],
                ctx[mc * 512:(mc + 1) * 512, :].rearrange("(j p) c -> p j c", p=P))
            ct = transpose_slab(cslab)
            pk = psp.tile([P, 512], F32, tag="p")
            for cc in range(CC):
                nc.tensor.matmul(pk[:], lhsT=wk_t[:, cc, :], rhs=ct[:, cc, :],
                                 start=(cc == 0), stop=(cc == CC - 1))
            kt = kt_pool.tile([P, 512], F32R, tag="kt")
            nc.scalar.activation(kt[:], pk[:], AF.Identity, bias=bk_t[:], scale=1.0)
            kt_tiles.append(kt)
            pv = psp.tile([P, 512], F32, tag="p")
            for cc in range(CC):
                nc.tensor.matmul(pv[:], lhsT=wv_t[:, cc, :], rhs=ct[:, cc, :],
                                 start=(cc == 0), stop=(cc == CC - 1))
            vt_sb = vt_pool.tile([P, 512], F32R, tag="vt")
            nc.vector.tensor_copy(vt_sb[:], pv[:])
            pvt = pst.tile([P, 512], F32R, tag="t")
            for j in range(4):
                nc.tensor.transpose(pvt[:, j * P:(j + 1) * P],
                                    vt_sb[:, j * P:(j + 1) * P], identr[:])
            for j in range(4):
                tm = mc * 4 + j
                b, mt = tm // MT, tm % MT
                vt_t = vn_pool.tile([P, 2, 65], F32R, tag="vn")
                nc.scalar.activation(
                    vt_t[:, :, 0:64],
                    pvt[:, j * P:(j + 1) * P].rearrange("p (a d) -> p a d", a=2),
                    AF.Identity, bias=0.0, scale=mf[:, b, mt:mt + 1])
                for h in range(2):
                    nc.vector.tensor_copy(vt_t[:, h, 64:65], mf[:, b, mt:mt + 1])
                vn_tiles.append(vt_t)
            if mc % 4 == 3:
                b = mc // 4
                if b == 3:
                    wo_half(0)
                attention_batch(b)
        wo_half(1)

        es2.close()

    nc.compile()
    return nc


def _get_nc():
    if "nc" not in _cached:
        _cached["nc"] = _build()
    return _cached["nc"]


def _prep_inputs(x, context, ctx_key_padding_mask, Wq, bq, Wk, bk, Wv, bv, Wo, bo):
    x = np.ascontiguousarray(np.asarray(x, dtype=np.float32).reshape(BN, C))
    ctx = np.ascontiguousarray(np.asarray(context, dtype=np.float32).reshape(BM, C))
    mask = np.ascontiguousarray(np.asarray(ctx_key_padding_mask, dtype=np.int32))
    Wq = np.asarray(Wq, dtype=np.float32)
    Wk = np.asarray(Wk, dtype=np.float32)
    Wv = np.asarray(Wv, dtype=np.float32)
    Wo = np.asarray(Wo, dtype=np.float32)
    bq = np.asarray(bq, dtype=np.float32)
    bk = np.asarray(bk, dtype=np.float32)
    bv = np.asarray(bv, dtype=np.float32)
    bo = np.asarray(bo, dtype=np.float32)
    # bv folds through the (row-stochastic) attention and Wo exactly:
    # out = (attn + bv) @ Wo.T + bo = attn @ Wo.T + (bo + Wo @ bv)
    bo_eff = (bo.astype(np.float64) + Wo.astype(np.float64) @ bv.astype(np.float64)
              ).astype(np.float32)
    bo_bc = np.ascontiguousarray(np.broadcast_to(bo_eff, (P, C)))
    wo_full = to_f32r(Wo.T)
    in_maps = []
    for r in range(R):
        sl = slice(r * DL, (r + 1) * DL)
        in_maps.append({
            "xin": x, "ctx": ctx, "mask": mask,
            "wq": to_f32r(Wq[sl, :].T), "wk": to_f32r(Wk[sl, :].T),
            "wv": to_f32r(Wv[sl, :].T), "wo": wo_full,
            "bq": np.ascontiguousarray(bq[sl].reshape(DL, 1)),
            "bk": np.ascontiguousarray(bk[sl].reshape(DL, 1)),
            "bo": bo_bc,
        })
    return in_maps


def _run(in_maps, **kwargs):
    from concourse.bass_utils import run_bass_kernel_spmd
    nc = _get_nc()
    return run_bass_kernel_spmd(nc, in_maps, list(range(R)), **kwargs)


def kernel(x, context, ctx_key_padding_mask, Wq, bq, Wk, bk, Wv, bv, Wo, bo):
    in_maps = _prep_inputs(x, context, ctx_key_padding_mask,
                           Wq, bq, Wk, bk, Wv, bv, Wo, bo)
    res = _run(in_maps).results
    out = np.empty((BN, C), dtype=np.float32)
    for r in range(R):
        o = res[r]["out"]          # [2, 128, C]: half h -> batch 2h + r//4,
        for h in range(2):         # rows (r%4)*128 ...
            b = 2 * h + r // 4
            row = b * N + (r % 4) * P
            out[row:row + P] = o[h]
    return np.ascontiguousarray(out.reshape(B, N, C))



# revision 19
# speedup vs baseline: 1.4329x; 1.4329x over previous
"""Cross-attention Trainium2 kernel, 8-way head-sharded (tensor parallel).

Strategy (per spec sharding_hint): split the 16 heads across the 8 cores
(2 heads / core) by slicing Wq/Wk/Wv column-wise (rows of the [out,in]
weight) and Wo row-wise. Each core computes q/k/v projections for its
128-dim slice, the masked-softmax attention for its 2 heads, then the
full-C output projection for a 1/8 slice of the (B*N) rows after an
AllToAll that redistributes the per-core attention outputs from
head-sharded to row-sharded. Host concatenates the 8 row slices.

v2 (this file): activations are pre-transposed AND pre-cast to bf16 on
the host (x^T [C,BN], ctx^T [C,BM]); the key-padding mask is folded into
ctx^T on the host (masked keys' columns zeroed), so the kernel has no
on-chip x/ctx transposes and no mask multiplies at all. All matmuls run
bf16 x bf16 with fp32 PSUM accumulation (1 cycle/row on the PE, same as
f32r, but transposes/DMA are 1-2x cheaper and no <256-column penalty).
Softmax denominators come from an extra mask column appended to V inside
the same PE accumulation as E@V; their reciprocal uses the fast DVE
approx op (~5x faster than nc.vector.reciprocal). Attention-phase
emission interleaves the next batch's K/V projection chunks between
score/AV matmul pairs so the PE never drains while the Act engine
(exp) catches up. End-to-end relative error ~4e-3 (tolerance 2e-2).
"""
import sys
sys.path.insert(0, '/opt/trn_rl_repo')

import numpy as np
import ml_dtypes

B, N, M, C, H, D = 4, 512, 2048, 1024, 16, 64
R = 8               # cores
DL = C // R         # per-core q/k/v slice width (2 heads x 64)
SCALE = D ** -0.5
BN, BM = B * N, B * M
CC = C // 128       # contraction chunks
MT = M // 128       # m-tiles per batch
TM = B * MT         # global m-tiles (64)
P = 128

_cached = {}


def _build():
    import concourse.tile as tile
    from concourse import bacc, mybir
    from concourse.masks import make_identity
    from contextlib import ExitStack

    F32 = mybir.dt.float32
    F32R = mybir.dt.float32r
    BF16 = mybir.dt.bfloat16
    AF = mybir.ActivationFunctionType
    OP = mybir.AluOpType

    nc = bacc.Bacc("TRN2", target_bir_lowering=False, debug=False, num_devices=R)

    xT_d = nc.dram_tensor("xT", [C, BN], BF16, kind="ExternalInput").ap()
    cT_d = nc.dram_tensor("cT", [C, BM], BF16, kind="ExternalInput").ap()
    mcol_d = nc.dram_tensor("mcol", [P, TM, 2], BF16, kind="ExternalInput").ap()
    wq_d = nc.dram_tensor("wq", [C, DL], BF16, kind="ExternalInput").ap()
    wk_d = nc.dram_tensor("wk", [C, DL], BF16, kind="ExternalInput").ap()
    wv_d = nc.dram_tensor("wv", [C, DL], BF16, kind="ExternalInput").ap()
    wo_d = nc.dram_tensor("wo", [C, C], BF16, kind="ExternalInput").ap()
    bq_d = nc.dram_tensor("bq", [DL, 1], F32, kind="ExternalInput").ap()
    bk_d = nc.dram_tensor("bk", [DL, 1], F32, kind="ExternalInput").ap()
    bo_d = nc.dram_tensor("bo", [P, C], F32, kind="ExternalInput").ap()
    out_d = nc.dram_tensor("out", [2, P, C], F32, kind="ExternalOutput").ap()

    with tile.TileContext(nc) as tc, ExitStack() as es:
        const = es.enter_context(tc.tile_pool(name="const", bufs=1))
        kt_pool = es.enter_context(tc.tile_pool(name="kt", bufs=8))
        vn_pool = es.enter_context(tc.tile_pool(name="vn", bufs=8))
        qt_pool = es.enter_context(tc.tile_pool(name="qt", bufs=4))
        av_pool = es.enter_context(tc.tile_pool(name="av", bufs=2))
        agp = es.enter_context(tc.tile_pool(name="agp", bufs=2))
        outp = es.enter_context(tc.tile_pool(name="outp", bufs=4))
        dram = es.enter_context(tc.tile_pool(name="dram", bufs=1, space="DRAM"))
        pst = es.enter_context(tc.tile_pool(name="pst", bufs=2, space="PSUM"))
        psp = es.enter_context(tc.tile_pool(name="psp", bufs=2, space="PSUM"))
        pss = es.enter_context(tc.tile_pool(name="pss", bufs=2, space="PSUM"))
        psa = es.enter_context(tc.tile_pool(name="psa", bufs=2, space="PSUM"))

        # ---- constants (weights on the gpsimd DMA queue, ahead of x) ----
        wq_t = const.tile([P, CC, DL], BF16, tag="wq")
        nc.gpsimd.dma_start(wq_t[:], wq_d.rearrange("(cc p) d -> p cc d", p=P))
        wk_t = const.tile([P, CC, DL], BF16, tag="wk")
        nc.gpsimd.dma_start(wk_t[:], wk_d.rearrange("(cc p) d -> p cc d", p=P))
        wv_t = const.tile([P, CC, DL], BF16, tag="wv")
        nc.gpsimd.dma_start(wv_t[:], wv_d.rearrange("(cc p) d -> p cc d", p=P))
        bq_t = const.tile([P, 1], F32, tag="bq")
        nc.gpsimd.dma_start(bq_t[:], bq_d[:])
        bk_t = const.tile([P, 1], F32, tag="bk")
        nc.gpsimd.dma_start(bk_t[:], bk_d[:])
        bo_t = const.tile([P, C], F32, tag="bo")
        nc.sync.dma_start(bo_t[:], bo_d[:])
        identf = const.tile([P, P], F32, tag="idf")
        make_identity(nc, identf[:])
        identr = const.tile([P, P], F32R, tag="idr")
        nc.vector.tensor_copy(identr[:], identf[:])
        ones_b = const.tile([1, 64], BF16, tag="ones")
        nc.gpsimd.memset(ones_b[:], 1.0)
        mcolt = const.tile([P, TM, 2], BF16, tag="mcol")
        nc.sync.dma_start(mcolt[:], mcol_d[:])

        es2 = ExitStack()
        xn_pool = es2.enter_context(tc.tile_pool(name="xn", bufs=2))
        cn_pool = es2.enter_context(tc.tile_pool(name="cn", bufs=4))
        vt_pool = es2.enter_context(tc.tile_pool(name="vt", bufs=2))
        e_pool = es2.enter_context(tc.tile_pool(name="e", bufs=6))
        nrm_pool = es2.enter_context(tc.tile_pool(name="nrm", bufs=4))

        kt_tiles = []
        vn_tiles = []
        qt_tiles = []
        av_tiles = []
        a2a_outs = []
        cslabs = {}

        # ---- Phase A: q^T per batch (x slabs on gpsimd queue) ----
        xslabs = []
        for b in range(B):
            xs = xn_pool.tile([P, CC, 512], BF16, tag="xs")
            nc.gpsimd.dma_start(
                xs[:], xT_d[:, b * 512:(b + 1) * 512]
                .rearrange("(cc p) n -> p cc n", p=P))
            xslabs.append(xs)

        # output projection weights + skew-absorbing barrier, behind the x
        # slabs on the gpsimd queue so they don't delay the first matmuls.
        wo_t = const.tile([P, CC, C], BF16, tag="wo")
        nc.gpsimd.dma_start(wo_t[:], wo_d.rearrange("(cc p) c -> p cc c", p=P))
        bar_in = dram.tile([1, 4], F32, tag="barin")
        bar_out = dram.tile([R, 4], F32, tag="barout")
        nc.sync.dma_start(bar_in[:], bq_d.rearrange("d o -> o d")[0:1, 0:4])
        nc.gpsimd.collective_compute(
            "AllGather", OP.bypass, replica_groups=[list(range(R))],
            ins=[bar_in.opt()], outs=[bar_out.opt()])

        def ctx_dma(mc):
            cs = cn_pool.tile([P, CC, 512], BF16, tag="cs")
            nc.sync.dma_start(
                cs[:], cT_d[:, mc * 512:(mc + 1) * 512]
                .rearrange("(cc p) n -> p cc n", p=P))
            cslabs[mc] = cs

        for mc in range(4):
            ctx_dma(mc)

        for b in range(B):
            pq = psp.tile([P, 512], F32, tag="p")
            for cc in range(CC):
                nc.tensor.matmul(pq[:], lhsT=wq_t[:, cc, :], rhs=xslabs[b][:, cc, :],
                                 start=(cc == 0), stop=(cc == CC - 1))
            qt = qt_pool.tile([P, 512], BF16, tag="qt")
            nc.scalar.activation(qt[:], pq[:], AF.Identity, bias=bq_t[:], scale=1.0)
            qt_tiles.append(qt)

        # ---- Phase B slab chunks (emitted inline or as attention filler) ----
        def slab_chunks(mc):
            def c_k():
                pk = psp.tile([P, 512], F32, tag="p", name=f"pk{mc}")
                for cc in range(CC):
                    nc.tensor.matmul(pk[:], lhsT=wk_t[:, cc, :],
                                     rhs=cslabs[mc][:, cc, :],
                                     start=(cc == 0), stop=(cc == CC - 1))
                kt = kt_pool.tile([P, 512], BF16, tag="kt")
                nc.scalar.activation(kt[:], pk[:], AF.Identity, bias=bk_t[:], scale=1.0)
                kt_tiles.append(kt)

            def c_v():
                pv = psp.tile([P, 512], F32, tag="p", name=f"pv{mc}")
                for cc in range(CC):
                    nc.tensor.matmul(pv[:], lhsT=wv_t[:, cc, :],
                                     rhs=cslabs[mc][:, cc, :],
                                     start=(cc == 0), stop=(cc == CC - 1))
                vt_sb = vt_pool.tile([P, 512], F32R, tag="vt")
                nc.vector.tensor_copy(vt_sb[:], pv[:])
                cslabs[mc] = None
                return vt_sb

            def c_t(vt_sb):
                pvt = pst.tile([P, 512], F32R, tag="t")
                for j in range(4):
                    nc.tensor.transpose(pvt[:, j * P:(j + 1) * P],
                                        vt_sb[:, j * P:(j + 1) * P], identr[:])
                vn = vn_pool.tile([P, 4, 2, 65], BF16, tag="vn")
                nc.vector.tensor_copy(
                    vn[:, :, :, 0:64],
                    pvt[:].rearrange("p (j a d) -> p j a d", j=4, a=2))
                nc.vector.tensor_copy(vn[:, :, :, 64:65],
                                      mcolt[:, mc * 4:(mc + 1) * 4, :])
                vn_tiles.append(vn)
                if mc + 4 < BM // 512:
                    ctx_dma(mc + 4)

            state = {}
            def chunk1(): state['v'] = None; c_k()
            def chunk2(): state['v'] = c_v()
            def chunk3(): c_t(state['v'])
            return [chunk1, chunk2, chunk3]

        # ---- attention per batch, with filler chunks interleaved ----
        def attention_batch(b, filler):
            pav = [psa.tile([P, 512], F32, tag="a", name=f"pav{_h}") for _h in range(2)]
            fi = 0
            for mt in range(MT):
                tm = b * MT + mt
                mc, j = tm // 4, tm % 4
                kt = kt_tiles[mc]
                vn = vn_tiles[mc]
                ps = [pss.tile([P, 512], F32, tag="s", name=f"ps{_h}") for _h in range(2)]
                for h in range(2):
                    nc.tensor.matmul(ps[h][:],
                                     lhsT=kt[h * 64:(h + 1) * 64, j * P:(j + 1) * P],
                                     rhs=qt_tiles[b][h * 64:(h + 1) * 64, :],
                                     start=True, stop=True)
                ee = []
                for h in range(2):
                    e = e_pool.tile([P, 512], BF16, tag="e")
                    nc.scalar.activation(e[:], ps[h][:], AF.Exp,
                                         bias=0.0, scale=float(SCALE))
                    ee.append(e)
                first, last = (mt == 0), (mt == MT - 1)
                for h in range(2):
                    # lhsT = [V_h | mask]: rows 0:64 = (EV)^T, row 64 = denom
                    nc.tensor.matmul(pav[h][0:65, :], lhsT=vn[:, j, h, :],
                                     rhs=ee[h][:], start=first, stop=last)
                if fi < len(filler):
                    filler[fi]()
                    fi += 1
            while fi < len(filler):
                filler[fi]()
                fi += 1
            avt = av_pool.tile([P, 512], BF16, tag="av")
            for h in range(2):
                den = nrm_pool.tile([1, 512], F32, tag="den")
                nc.vector.tensor_copy(den[:], pav[h][64:65, :])
                rec = nrm_pool.tile([1, 512], F32, tag="rec")
                nc.vector.reciprocal_approx_fast(rec[:], den[:])
                rec_b = nrm_pool.tile([1, 512], BF16, tag="recb")
                nc.vector.tensor_copy(rec_b[:], rec[:])
                pb = psp.tile([P, 512], F32, tag="p")
                nc.tensor.matmul(pb[0:64, :], lhsT=ones_b[:], rhs=rec_b[:],
                                 start=True, stop=True)
                bc = nrm_pool.tile([64, 512], F32, tag="bc")
                nc.vector.tensor_copy(bc[:], pb[0:64, :])
                nc.vector.scalar_tensor_tensor(
                    out=avt[h * 64:(h + 1) * 64, :], in0=pav[h][0:64, :],
                    scalar=1.0, in1=bc[:], op0=OP.mult, op1=OP.mult)
            av_tiles.append(avt)
            if b % 2 == 1:
                hh = b // 2
                a2a_in = dram.tile([R, P, P], BF16, name=f"a2ai{hh}")
                a2a_out = dram.tile([R, P, P], BF16, name=f"a2ao{hh}")
                for j in range(R):
                    src = av_tiles[2 * hh + j // 4]
                    nc.scalar.dma_start(a2a_in[j, :, :],
                                        src[:, (j % 4) * P:(j % 4 + 1) * P])
                nc.gpsimd.collective_compute(
                    "AllToAll", OP.bypass, replica_groups=[list(range(R))],
                    ins=[a2a_in.opt()], outs=[a2a_out.opt()])
                a2a_outs.append(a2a_out)

        def wo_half(h):
            """Deferred behind the a2a so the collective completes off the
            critical path (in-order engine queues)."""
            agt = agp.tile([P, CC, P], BF16, name=f"agt{h}", tag="ag")
            nc.scalar.dma_start(agt[:], a2a_outs[h].rearrange("i p n -> p i n"))
            for ch in range(2):
                po = psp.tile([P, 512], F32, tag="p", name=f"po{h}_{ch}")
                for cc in range(CC):
                    nc.tensor.matmul(po[:], lhsT=agt[:, cc, :],
                                     rhs=wo_t[:, cc, ch * 512:(ch + 1) * 512],
                                     start=(cc == 0), stop=(cc == CC - 1))
                ob = outp.tile([P, 512], F32, tag="ob", name=f"ob{h}_{ch}")
                nc.vector.scalar_tensor_tensor(
                    out=ob[:], in0=po[:], scalar=1.0,
                    in1=bo_t[:, ch * 512:(ch + 1) * 512],
                    op0=OP.mult, op1=OP.add)
                nc.scalar.dma_start(out_d[h, :, ch * 512:(ch + 1) * 512], ob[:])

        # slabs 0-3 inline (batch 0's keys), then attention batches with the
        # next batch's slab chunks (or wo_half(0)) as PE filler.
        INTERLEAVE = True
        for mc in range(4):
            for c in slab_chunks(mc):
                c()
        for b in range(B):
            if b < 3:
                filler = []
                for k in range(4):
                    filler += slab_chunks(4 * (b + 1) + k)
            else:
                filler = [lambda: wo_half(0)]
            if not INTERLEAVE:
                for f in filler:
                    f()
                filler = []
            attention_batch(b, filler)
        wo_half(1)

        es2.close()

    nc.compile()
    return nc


def _get_nc():
    if "nc" not in _cached:
        _cached["nc"] = _build()
    return _cached["nc"]


def _prep_inputs(x, context, ctx_key_padding_mask, Wq, bq, Wk, bk, Wv, bv, Wo, bo):
    bf16 = ml_dtypes.bfloat16
    x = np.asarray(x, dtype=np.float32).reshape(BN, C)
    ctx = np.asarray(context, dtype=np.float32).reshape(BM, C)
    mask = np.asarray(ctx_key_padding_mask, dtype=np.int32)
    Wq = np.asarray(Wq, dtype=np.float32)
    Wk = np.asarray(Wk, dtype=np.float32)
    Wv = np.asarray(Wv, dtype=np.float32)
    Wo = np.asarray(Wo, dtype=np.float32)
    bq = np.asarray(bq, dtype=np.float32)
    bk = np.asarray(bk, dtype=np.float32)
    bv = np.asarray(bv, dtype=np.float32)
    bo = np.asarray(bo, dtype=np.float32)
    # bv folds through the (row-stochastic) attention and Wo exactly:
    # out = (attn + bv) @ Wo.T + bo = attn @ Wo.T + (bo + Wo @ bv)
    bo_eff = (bo.astype(np.float64) + Wo.astype(np.float64) @ bv.astype(np.float64)
              ).astype(np.float32)
    bo_bc = np.ascontiguousarray(np.broadcast_to(bo_eff, (P, C)))
    # key-padding mask folded into ctx^T: masked keys' columns become 0, so
    # v is masked for free; their k rows are bk-only but those E values are
    # multiplied by the zero mask column in both softmax sums.
    mf = (mask != 0).astype(np.float32).reshape(BM)      # [BM]
    xT = np.ascontiguousarray(x.T).astype(bf16)          # [C, BN]
    cT = np.ascontiguousarray((ctx * mf[:, None]).T).astype(bf16)  # [C, BM]
    # mcol[p, tm, h] = mask value for key tm*128+p (same for both heads)
    mcol = np.ascontiguousarray(
        np.broadcast_to(mf.reshape(TM, P).T[:, :, None], (P, TM, 2))).astype(bf16)
    wo_full = np.ascontiguousarray(Wo.T).astype(bf16)
    in_maps = []
    for r in range(R):
        sl = slice(r * DL, (r + 1) * DL)
        in_maps.append({
            "xT": xT, "cT": cT, "mcol": mcol,
            "wq": np.ascontiguousarray(Wq[sl, :].T).astype(bf16),
            "wk": np.ascontiguousarray(Wk[sl, :].T).astype(bf16),
            "wv": np.ascontiguousarray(Wv[sl, :].T).astype(bf16),
            "wo": wo_full,
            "bq": np.ascontiguousarray(bq[sl].reshape(DL, 1)),
            "bk": np.ascontiguousarray(bk[sl].reshape(DL, 1)),
            "bo": bo_bc,
        })
    return in_maps


def _run(in_maps, **kwargs):
    from concourse.bass_utils import run_bass_kernel_spmd
    nc = _get_nc()
    return run_bass_kernel_spmd(nc, in_maps, list(range(R)), **kwargs)


def kernel(x, context, ctx_key_padding_mask, Wq, bq, Wk, bk, Wv, bv, Wo, bo):
    in_maps = _prep_inputs(x, context, ctx_key_padding_mask,
                           Wq, bq, Wk, bk, Wv, bv, Wo, bo)
    res = _run(in_maps).results
    out = np.empty((BN, C), dtype=np.float32)
    for r in range(R):
        o = res[r]["out"]          # [2, 128, C]: half h -> batch 2h + r//4,
        for h in range(2):         # rows (r%4)*128 ...
            b = 2 * h + r // 4
            row = b * N + (r % 4) * P
            out[row:row + P] = o[h]
    return np.ascontiguousarray(out.reshape(B, N, C))


# revision 20
# speedup vs baseline: 1.5369x; 1.0725x over previous
"""Cross-attention Trainium2 kernel, 8-way head-sharded (tensor parallel).

Strategy (per spec sharding_hint): split the 16 heads across the 8 cores
(2 heads / core) by slicing Wq/Wk/Wv column-wise (rows of the [out,in]
weight) and Wo row-wise. Each core computes q/k/v projections for its
128-dim slice, the masked-softmax attention for its 2 heads, then the
full-C output projection for a 1/8 slice of the (B*N) rows after an
AllToAll that redistributes the per-core attention outputs from
head-sharded to row-sharded. Host concatenates the 8 row slices.

v2 (this file): activations are pre-transposed AND pre-cast to bf16 on
the host (x^T [C,BN], ctx^T [C,BM]); the key-padding mask is folded into
ctx^T on the host (masked keys' columns zeroed), so the kernel has no
on-chip x/ctx transposes and no mask multiplies at all. All matmuls run
bf16 x bf16 with fp32 PSUM accumulation (1 cycle/row on the PE, same as
f32r, but transposes/DMA are 1-2x cheaper and no <256-column penalty).
Softmax denominators come from an extra mask column appended to V inside
the same PE accumulation as E@V; their reciprocal uses the fast DVE
approx op (~5x faster than nc.vector.reciprocal). Attention-phase
emission interleaves the next batch's K/V projection chunks between
score/AV matmul pairs so the PE never drains while the Act engine
(exp) catches up. End-to-end relative error ~4e-3 (tolerance 2e-2).
"""
import sys
sys.path.insert(0, '/opt/trn_rl_repo')

import numpy as np
import ml_dtypes

B, N, M, C, H, D = 4, 512, 2048, 1024, 16, 64
R = 8               # cores
DL = C // R         # per-core q/k/v slice width (2 heads x 64)
SCALE = D ** -0.5
BN, BM = B * N, B * M
CC = C // 128       # contraction chunks
MT = M // 128       # m-tiles per batch
TM = B * MT         # global m-tiles (64)
P = 128

_cached = {}


def _build():
    import concourse.tile as tile
    from concourse import bacc, mybir
    from concourse.masks import make_identity
    from contextlib import ExitStack

    F32 = mybir.dt.float32
    F32R = mybir.dt.float32r
    BF16 = mybir.dt.bfloat16
    AF = mybir.ActivationFunctionType
    OP = mybir.AluOpType

    nc = bacc.Bacc("TRN2", target_bir_lowering=False, debug=False, num_devices=R)

    xT_d = nc.dram_tensor("xT", [C, BN], BF16, kind="ExternalInput").ap()
    cT_d = nc.dram_tensor("cT", [C, BM], BF16, kind="ExternalInput").ap()
    mcol_d = nc.dram_tensor("mcol", [P, TM, 2], BF16, kind="ExternalInput").ap()
    wq_d = nc.dram_tensor("wq", [C, DL], BF16, kind="ExternalInput").ap()
    wk_d = nc.dram_tensor("wk", [C, DL], BF16, kind="ExternalInput").ap()
    wv_d = nc.dram_tensor("wv", [C, DL], BF16, kind="ExternalInput").ap()
    wo_d = nc.dram_tensor("wo", [C, C], BF16, kind="ExternalInput").ap()
    bq_d = nc.dram_tensor("bq", [DL, 1], F32, kind="ExternalInput").ap()
    bk_d = nc.dram_tensor("bk", [DL, 1], F32, kind="ExternalInput").ap()
    bo_d = nc.dram_tensor("bo", [P, C], F32, kind="ExternalInput").ap()
    out_d = nc.dram_tensor("out", [2, P, C], BF16, kind="ExternalOutput").ap()

    with tile.TileContext(nc) as tc, ExitStack() as es:
        const = es.enter_context(tc.tile_pool(name="const", bufs=1))
        kt_pool = es.enter_context(tc.tile_pool(name="kt", bufs=8))
        vn_pool = es.enter_context(tc.tile_pool(name="vn", bufs=8))
        qt_pool = es.enter_context(tc.tile_pool(name="qt", bufs=4))
        av_pool = es.enter_context(tc.tile_pool(name="av", bufs=2))
        agp = es.enter_context(tc.tile_pool(name="agp", bufs=2))
        outp = es.enter_context(tc.tile_pool(name="outp", bufs=4))
        dram = es.enter_context(tc.tile_pool(name="dram", bufs=1, space="DRAM"))
        pst = es.enter_context(tc.tile_pool(name="pst", bufs=2, space="PSUM"))
        psp = es.enter_context(tc.tile_pool(name="psp", bufs=2, space="PSUM"))
        pss = es.enter_context(tc.tile_pool(name="pss", bufs=2, space="PSUM"))
        psa = es.enter_context(tc.tile_pool(name="psa", bufs=2, space="PSUM"))

        # ---- constants: one priority-ordered stream on the sync queue so
        # early ctx slabs are never starved by later/larger loads ----
        wq_t = const.tile([P, CC, DL], BF16, tag="wq")
        nc.sync.dma_start(wq_t[:], wq_d.rearrange("(cc p) d -> p cc d", p=P))
        wk_t = const.tile([P, CC, DL], BF16, tag="wk")
        nc.sync.dma_start(wk_t[:], wk_d.rearrange("(cc p) d -> p cc d", p=P))
        wv_t = const.tile([P, CC, DL], BF16, tag="wv")
        nc.sync.dma_start(wv_t[:], wv_d.rearrange("(cc p) d -> p cc d", p=P))
        bq_t = const.tile([P, 1], F32, tag="bq")
        nc.sync.dma_start(bq_t[:], bq_d[:])
        bk_t = const.tile([P, 1], F32, tag="bk")
        nc.sync.dma_start(bk_t[:], bk_d[:])
        mcolt = const.tile([P, TM, 2], BF16, tag="mcol")
        nc.sync.dma_start(mcolt[:], mcol_d[:])
        bo_t = const.tile([P, C], F32, tag="bo")
        nc.scalar.dma_start(bo_t[:], bo_d[:])
        identf = const.tile([P, P], F32, tag="idf")
        make_identity(nc, identf[:])
        identr = const.tile([P, P], F32R, tag="idr")
        nc.vector.tensor_copy(identr[:], identf[:])
        ones_b = const.tile([1, 64], BF16, tag="ones")
        nc.gpsimd.memset(ones_b[:], 1.0)
        wo_t = const.tile([P, CC, C], BF16, tag="wo")

        es2 = ExitStack()
        xn_pool = es2.enter_context(tc.tile_pool(name="xn", bufs=2))
        cn_pool = es2.enter_context(tc.tile_pool(name="cn", bufs=4))
        vt_pool = es2.enter_context(tc.tile_pool(name="vt", bufs=2))
        e_pool = es2.enter_context(tc.tile_pool(name="e", bufs=6))
        nrm_pool = es2.enter_context(tc.tile_pool(name="nrm", bufs=4))

        kt_tiles = []
        vn_tiles = []
        qt_tiles = []
        av_tiles = []
        a2a_outs = []
        cslabs = {}

        # ---- DMA stream, consumption order: ctx0 ctx1 x0 ctx2 x1 ctx3 x2
        # x3 ctx4.. (barrier + wo_t wedge in behind the early slabs) ----
        xslabs = []

        def x_dma(b):
            xs = xn_pool.tile([P, CC, 512], BF16, tag="xs")
            nc.sync.dma_start(
                xs[:], xT_d[:, b * 512:(b + 1) * 512]
                .rearrange("(cc p) n -> p cc n", p=P))
            xslabs.append(xs)

        def ctx_dma(mc):
            cs = cn_pool.tile([P, CC, 512], BF16, tag="cs")
            nc.sync.dma_start(
                cs[:], cT_d[:, mc * 512:(mc + 1) * 512]
                .rearrange("(cc p) n -> p cc n", p=P))
            cslabs[mc] = cs

        ctx_dma(0)
        ctx_dma(1)
        x_dma(0)
        ctx_dma(2)
        x_dma(1)
        ctx_dma(3)
        x_dma(2)
        x_dma(3)

        bar_in = dram.tile([1, 4], F32, tag="barin")
        bar_out = dram.tile([R, 4], F32, tag="barout")
        nc.scalar.dma_start(bar_in[:], bq_d.rearrange("d o -> o d")[0:1, 0:4])
        nc.gpsimd.collective_compute(
            "AllGather", OP.bypass, replica_groups=[list(range(R))],
            ins=[bar_in.opt()], outs=[bar_out.opt()])

        def q_proj(b):
            pq = psp.tile([P, 512], F32, tag="p")
            for cc in range(CC):
                nc.tensor.matmul(pq[:], lhsT=wq_t[:, cc, :], rhs=xslabs[b][:, cc, :],
                                 start=(cc == 0), stop=(cc == CC - 1))
            qt = qt_pool.tile([P, 512], BF16, tag="qt")
            nc.scalar.activation(qt[:], pq[:], AF.Identity, bias=bq_t[:], scale=1.0)
            qt_tiles.append(qt)

        # ---- Phase B slab chunks (emitted inline or as attention filler) ----
        def slab_chunks(mc):
            def c_k():
                pk = psp.tile([P, 512], F32, tag="p", name=f"pk{mc}")
                for cc in range(CC):
                    nc.tensor.matmul(pk[:], lhsT=wk_t[:, cc, :],
                                     rhs=cslabs[mc][:, cc, :],
                                     start=(cc == 0), stop=(cc == CC - 1))
                kt = kt_pool.tile([P, 512], BF16, tag="kt")
                nc.scalar.activation(kt[:], pk[:], AF.Identity, bias=bk_t[:], scale=1.0)
                kt_tiles.append(kt)

            def c_v():
                pv = psp.tile([P, 512], F32, tag="p", name=f"pv{mc}")
                for cc in range(CC):
                    nc.tensor.matmul(pv[:], lhsT=wv_t[:, cc, :],
                                     rhs=cslabs[mc][:, cc, :],
                                     start=(cc == 0), stop=(cc == CC - 1))
                vt_sb = vt_pool.tile([P, 512], F32R, tag="vt")
                nc.vector.tensor_copy(vt_sb[:], pv[:])
                cslabs[mc] = None
                return vt_sb

            def c_t(vt_sb):
                pvt = pst.tile([P, 512], F32R, tag="t")
                for j in range(4):
                    nc.tensor.transpose(pvt[:, j * P:(j + 1) * P],
                                        vt_sb[:, j * P:(j + 1) * P], identr[:])
                vn = vn_pool.tile([P, 4, 2, 65], BF16, tag="vn")
                nc.vector.tensor_copy(
                    vn[:, :, :, 0:64],
                    pvt[:].rearrange("p (j a d) -> p j a d", j=4, a=2))
                nc.vector.tensor_copy(vn[:, :, :, 64:65],
                                      mcolt[:, mc * 4:(mc + 1) * 4, :])
                vn_tiles.append(vn)

            state = {}
            def chunk1(): state['v'] = None; c_k()
            def chunk2(): state['v'] = c_v()
            def chunk3(): c_t(state['v'])
            return [chunk1, chunk2, chunk3]

        # ---- attention per batch, with filler chunks interleaved ----
        def attention_batch(b, filler):
            pav = [psa.tile([P, 512], F32, tag="a", name=f"pav{_h}") for _h in range(2)]
            fi = 0
            for mt in range(MT):
                tm = b * MT + mt
                mc, j = tm // 4, tm % 4
                kt = kt_tiles[mc]
                vn = vn_tiles[mc]
                ps = [pss.tile([P, 512], F32, tag="s", name=f"ps{_h}") for _h in range(2)]
                for h in range(2):
                    nc.tensor.matmul(ps[h][:],
                                     lhsT=kt[h * 64:(h + 1) * 64, j * P:(j + 1) * P],
                                     rhs=qt_tiles[b][h * 64:(h + 1) * 64, :],
                                     start=True, stop=True)
                ee = []
                for h in range(2):
                    e = e_pool.tile([P, 512], BF16, tag="e")
                    nc.scalar.activation(e[:], ps[h][:], AF.Exp,
                                         bias=0.0, scale=float(SCALE))
                    ee.append(e)
                if fi < len(filler):
                    filler[fi]()
                    fi += 1
                first, last = (mt == 0), (mt == MT - 1)
                for h in range(2):
                    # lhsT = [V_h | mask]: rows 0:64 = (EV)^T, row 64 = denom
                    nc.tensor.matmul(pav[h][0:65, :], lhsT=vn[:, j, h, :],
                                     rhs=ee[h][:], start=first, stop=last)
            while fi < len(filler):
                filler[fi]()
                fi += 1
            avt = av_pool.tile([P, 512], BF16, tag="av")
            for h in range(2):
                den = nrm_pool.tile([1, 512], F32, tag="den")
                nc.vector.tensor_copy(den[:], pav[h][64:65, :])
                rec = nrm_pool.tile([1, 512], F32, tag="rec")
                nc.vector.reciprocal_approx_fast(rec[:], den[:])
                rec_b = nrm_pool.tile([1, 512], BF16, tag="recb")
                nc.vector.tensor_copy(rec_b[:], rec[:])
                pb = psp.tile([P, 512], F32, tag="p")
                nc.tensor.matmul(pb[0:64, :], lhsT=ones_b[:], rhs=rec_b[:],
                                 start=True, stop=True)
                bc = nrm_pool.tile([64, 512], F32, tag="bc")
                nc.vector.tensor_copy(bc[:], pb[0:64, :])
                nc.vector.scalar_tensor_tensor(
                    out=avt[h * 64:(h + 1) * 64, :], in0=pav[h][0:64, :],
                    scalar=1.0, in1=bc[:], op0=OP.mult, op1=OP.mult)
            av_tiles.append(avt)
            if b % 2 == 1:
                hh = b // 2
                a2a_in = dram.tile([R, P, P], BF16, name=f"a2ai{hh}")
                a2a_out = dram.tile([R, P, P], BF16, name=f"a2ao{hh}")
                for j in range(R):
                    src = av_tiles[2 * hh + j // 4]
                    nc.scalar.dma_start(a2a_in[j, :, :],
                                        src[:, (j % 4) * P:(j % 4 + 1) * P])
                nc.gpsimd.collective_compute(
                    "AllToAll", OP.bypass, replica_groups=[list(range(R))],
                    ins=[a2a_in.opt()], outs=[a2a_out.opt()])
                a2a_outs.append(a2a_out)

        def wo_half(h):
            """Deferred behind the a2a so the collective completes off the
            critical path (in-order engine queues)."""
            agt = agp.tile([P, CC, P], BF16, name=f"agt{h}", tag="ag")
            nc.scalar.dma_start(agt[:], a2a_outs[h].rearrange("i p n -> p i n"))
            for ch in range(2):
                po = psp.tile([P, 512], F32, tag="p", name=f"po{h}_{ch}")
                for cc in range(CC):
                    nc.tensor.matmul(po[:], lhsT=agt[:, cc, :],
                                     rhs=wo_t[:, cc, ch * 512:(ch + 1) * 512],
                                     start=(cc == 0), stop=(cc == CC - 1))
                ob = outp.tile([P, 512], BF16, tag="ob", name=f"ob{h}_{ch}")
                nc.vector.scalar_tensor_tensor(
                    out=ob[:], in0=po[:], scalar=1.0,
                    in1=bo_t[:, ch * 512:(ch + 1) * 512],
                    op0=OP.mult, op1=OP.add)
                nc.scalar.dma_start(out_d[h, :, ch * 512:(ch + 1) * 512], ob[:])

        # slabs 0-3 inline (batch 0's keys), then attention batches with the
        # next batch's slab chunks (or wo_half(0)) as PE filler.
        # prologue: slabs 0-3 + q-projections, ctx dmas rolling 4 ahead
        chunks0 = slab_chunks(0)
        chunks1 = slab_chunks(1)
        chunks0[0](); chunks0[1](); chunks0[2]()
        ctx_dma(4)
        q_proj(0)
        chunks1[0](); chunks1[1](); chunks1[2]()
        ctx_dma(5)
        q_proj(1)
        for c in slab_chunks(2):
            c()
        ctx_dma(6)
        q_proj(2)
        for c in slab_chunks(3):
            c()
        ctx_dma(7)
        q_proj(3)
        wo_dmaed = False
        for b in range(B):
            if b < 3:
                filler = []
                for k in range(4):
                    mc = 4 * (b + 1) + k
                    cks = slab_chunks(mc)
                    def mk_tail(ck3, nxt):
                        def f():
                            ck3()
                            if nxt is not None:
                                ctx_dma(nxt)
                        return f
                    nxt = mc + 4 if mc + 4 < 16 else None
                    filler += [cks[0], cks[1], mk_tail(cks[2], nxt)]
            else:
                filler = [lambda: wo_half(0)]
            attention_batch(b, filler)
            if b == 1 and not wo_dmaed:
                # wo arrives behind all ctx slabs, ahead of first use
                nc.sync.dma_start(wo_t[:], wo_d.rearrange("(cc p) c -> p cc c", p=P))
                wo_dmaed = True
        wo_half(1)

        es2.close()

    nc.compile()
    return nc


def _get_nc():
    if "nc" not in _cached:
        _cached["nc"] = _build()
    return _cached["nc"]


def _prep_inputs(x, context, ctx_key_padding_mask, Wq, bq, Wk, bk, Wv, bv, Wo, bo):
    bf16 = ml_dtypes.bfloat16
    x = np.asarray(x, dtype=np.float32).reshape(BN, C)
    ctx = np.asarray(context, dtype=np.float32).reshape(BM, C)
    mask = np.asarray(ctx_key_padding_mask, dtype=np.int32)
    Wq = np.asarray(Wq, dtype=np.float32)
    Wk = np.asarray(Wk, dtype=np.float32)
    Wv = np.asarray(Wv, dtype=np.float32)
    Wo = np.asarray(Wo, dtype=np.float32)
    bq = np.asarray(bq, dtype=np.float32)
    bk = np.asarray(bk, dtype=np.float32)
    bv = np.asarray(bv, dtype=np.float32)
    bo = np.asarray(bo, dtype=np.float32)
    # bv folds through the (row-stochastic) attention and Wo exactly:
    # out = (attn + bv) @ Wo.T + bo = attn @ Wo.T + (bo + Wo @ bv)
    bo_eff = (bo.astype(np.float64) + Wo.astype(np.float64) @ bv.astype(np.float64)
              ).astype(np.float32)
    bo_bc = np.ascontiguousarray(np.broadcast_to(bo_eff, (P, C)))
    # key-padding mask folded into ctx^T: masked keys' columns become 0, so
    # v is masked for free; their k rows are bk-only but those E values are
    # multiplied by the zero mask column in both softmax sums.
    mf = (mask != 0).astype(np.float32).reshape(BM)      # [BM]
    xT = np.ascontiguousarray(x.T).astype(bf16)          # [C, BN]
    cT = np.ascontiguousarray((ctx * mf[:, None]).T).astype(bf16)  # [C, BM]
    # mcol[p, tm, h] = mask value for key tm*128+p (same for both heads)
    mcol = np.ascontiguousarray(
        np.broadcast_to(mf.reshape(TM, P).T[:, :, None], (P, TM, 2))).astype(bf16)
    wo_full = np.ascontiguousarray(Wo.T).astype(bf16)
    in_maps = []
    for r in range(R):
        sl = slice(r * DL, (r + 1) * DL)
        in_maps.append({
            "xT": xT, "cT": cT, "mcol": mcol,
            "wq": np.ascontiguousarray(Wq[sl, :].T).astype(bf16),
            "wk": np.ascontiguousarray(Wk[sl, :].T).astype(bf16),
            "wv": np.ascontiguousarray(Wv[sl, :].T).astype(bf16),
            "wo": wo_full,
            "bq": np.ascontiguousarray(bq[sl].reshape(DL, 1)),
            "bk": np.ascontiguousarray(bk[sl].reshape(DL, 1)),
            "bo": bo_bc,
        })
    return in_maps


def _run(in_maps, **kwargs):
    from concourse.bass_utils import run_bass_kernel_spmd
    nc = _get_nc()
    return run_bass_kernel_spmd(nc, in_maps, list(range(R)), **kwargs)


def kernel(x, context, ctx_key_padding_mask, Wq, bq, Wk, bk, Wv, bv, Wo, bo):
    in_maps = _prep_inputs(x, context, ctx_key_padding_mask,
                           Wq, bq, Wk, bk, Wv, bv, Wo, bo)
    res = _run(in_maps).results
    out = np.empty((BN, C), dtype=np.float32)
    for r in range(R):
        o = res[r]["out"]          # [2, 128, C]: half h -> batch 2h + r//4,
        for h in range(2):         # rows (r%4)*128 ...
            b = 2 * h + r // 4
            row = b * N + (r % 4) * P
            out[row:row + P] = o[h]
    return np.ascontiguousarray(out.reshape(B, N, C))


# revision 21
# speedup vs baseline: 1.5991x; 1.0405x over previous
"""Cross-attention Trainium2 kernel, 8-way head-sharded (tensor parallel).

Strategy (per spec sharding_hint): split the 16 heads across the 8 cores
(2 heads / core) by slicing Wq/Wk/Wv column-wise (rows of the [out,in]
weight) and Wo row-wise. Each core computes q/k/v projections for its
128-dim slice, the masked-softmax attention for its 2 heads, then the
full-C output projection for a 1/8 slice of the (B*N) rows after an
AllToAll that redistributes the per-core attention outputs from
head-sharded to row-sharded. Host concatenates the 8 row slices.

v2 (this file): activations are pre-transposed AND pre-cast to bf16 on
the host (x^T [C,BN], ctx^T [C,BM]); the key-padding mask is folded into
ctx^T on the host (masked keys' columns zeroed), so the kernel has no
on-chip x/ctx transposes and no mask multiplies at all. All matmuls run
bf16 x bf16 with fp32 PSUM accumulation (1 cycle/row on the PE, same as
f32r, but transposes/DMA are 1-2x cheaper and no <256-column penalty).
Softmax denominators come from an extra mask column appended to V inside
the same PE accumulation as E@V; their reciprocal uses the fast DVE
approx op (~5x faster than nc.vector.reciprocal). Attention-phase
emission interleaves the next batch's K/V projection chunks between
score/AV matmul pairs so the PE never drains while the Act engine
(exp) catches up. End-to-end relative error ~4e-3 (tolerance 2e-2).
"""
import sys
sys.path.insert(0, '/opt/trn_rl_repo')

import numpy as np
import ml_dtypes

B, N, M, C, H, D = 4, 512, 2048, 1024, 16, 64
R = 8               # cores
DL = C // R         # per-core q/k/v slice width (2 heads x 64)
SCALE = D ** -0.5
BN, BM = B * N, B * M
CC = C // 128       # contraction chunks
MT = M // 128       # m-tiles per batch
TM = B * MT         # global m-tiles (64)
P = 128

_cached = {}


def _build():
    import concourse.tile as tile
    from concourse import bacc, mybir
    from concourse.masks import make_identity
    from contextlib import ExitStack

    F32 = mybir.dt.float32
    F32R = mybir.dt.float32r
    BF16 = mybir.dt.bfloat16
    AF = mybir.ActivationFunctionType
    OP = mybir.AluOpType

    nc = bacc.Bacc("TRN2", target_bir_lowering=False, debug=False, num_devices=R)

    xT_d = nc.dram_tensor("xT", [C, BN], BF16, kind="ExternalInput").ap()
    cT_d = nc.dram_tensor("cT", [C, BM], BF16, kind="ExternalInput").ap()
    mcol_d = nc.dram_tensor("mcol", [P, TM, 2], BF16, kind="ExternalInput").ap()
    wq_d = nc.dram_tensor("wq", [C, DL], BF16, kind="ExternalInput").ap()
    wk_d = nc.dram_tensor("wk", [C, DL], BF16, kind="ExternalInput").ap()
    wv_d = nc.dram_tensor("wv", [C, DL], BF16, kind="ExternalInput").ap()
    wo_d = nc.dram_tensor("wo", [C, C], BF16, kind="ExternalInput").ap()
    bq_d = nc.dram_tensor("bq", [DL, 1], F32, kind="ExternalInput").ap()
    bk_d = nc.dram_tensor("bk", [DL, 1], F32, kind="ExternalInput").ap()
    bo_d = nc.dram_tensor("bo", [P, C], F32, kind="ExternalInput").ap()
    out_d = nc.dram_tensor("out", [2, P, C], BF16, kind="ExternalOutput").ap()

    with tile.TileContext(nc) as tc, ExitStack() as es:
        const = es.enter_context(tc.tile_pool(name="const", bufs=1))
        kt_pool = es.enter_context(tc.tile_pool(name="kt", bufs=8))
        vn_pool = es.enter_context(tc.tile_pool(name="vn", bufs=8))
        qt_pool = es.enter_context(tc.tile_pool(name="qt", bufs=4))
        av_pool = es.enter_context(tc.tile_pool(name="av", bufs=2))
        agp = es.enter_context(tc.tile_pool(name="agp", bufs=2))
        outp = es.enter_context(tc.tile_pool(name="outp", bufs=4))
        dram = es.enter_context(tc.tile_pool(name="dram", bufs=1, space="DRAM"))
        pst = es.enter_context(tc.tile_pool(name="pst", bufs=2, space="PSUM"))
        psp = es.enter_context(tc.tile_pool(name="psp", bufs=2, space="PSUM"))
        pss = es.enter_context(tc.tile_pool(name="pss", bufs=2, space="PSUM"))
        psa = es.enter_context(tc.tile_pool(name="psa", bufs=2, space="PSUM"))

        # ---- constants: one priority-ordered stream on the sync queue so
        # early ctx slabs are never starved by later/larger loads ----
        wq_t = const.tile([P, CC, DL], BF16, tag="wq")
        nc.sync.dma_start(wq_t[:], wq_d.rearrange("(cc p) d -> p cc d", p=P))
        wk_t = const.tile([P, CC, DL], BF16, tag="wk")
        nc.sync.dma_start(wk_t[:], wk_d.rearrange("(cc p) d -> p cc d", p=P))
        wv_t = const.tile([P, CC, DL], BF16, tag="wv")
        nc.sync.dma_start(wv_t[:], wv_d.rearrange("(cc p) d -> p cc d", p=P))
        bq_t = const.tile([P, 1], F32, tag="bq")
        nc.sync.dma_start(bq_t[:], bq_d[:])
        bk_t = const.tile([P, 1], F32, tag="bk")
        nc.sync.dma_start(bk_t[:], bk_d[:])
        mcolt = const.tile([P, TM, 2], BF16, tag="mcol")
        nc.sync.dma_start(mcolt[:], mcol_d[:])
        bo_t = const.tile([P, C], F32, tag="bo")
        nc.scalar.dma_start(bo_t[:], bo_d[:])
        identf = const.tile([P, P], F32, tag="idf")
        make_identity(nc, identf[:])
        identr = const.tile([P, P], F32R, tag="idr")
        nc.vector.tensor_copy(identr[:], identf[:])
        ones_b = const.tile([1, 64], BF16, tag="ones")
        nc.gpsimd.memset(ones_b[:], 1.0)
        wo_t = const.tile([P, CC, C], BF16, tag="wo")

        es2 = ExitStack()
        xn_pool = es2.enter_context(tc.tile_pool(name="xn", bufs=2))
        cn_pool = es2.enter_context(tc.tile_pool(name="cn", bufs=4))
        vt_pool = es2.enter_context(tc.tile_pool(name="vt", bufs=2))
        e_pool = es2.enter_context(tc.tile_pool(name="e", bufs=6))
        nrm_pool = es2.enter_context(tc.tile_pool(name="nrm", bufs=4))

        kt_tiles = []
        vn_tiles = []
        qt_tiles = []
        av_tiles = []
        a2a_ins = []
        a2a_outs = []
        cslabs = {}

        # ---- DMA stream, consumption order: ctx0 ctx1 x0 ctx2 x1 ctx3 x2
        # x3 ctx4.. (barrier + wo_t wedge in behind the early slabs) ----
        xslabs = []

        def x_dma(b):
            xs = xn_pool.tile([P, CC, 512], BF16, tag="xs")
            nc.sync.dma_start(
                xs[:], xT_d[:, b * 512:(b + 1) * 512]
                .rearrange("(cc p) n -> p cc n", p=P))
            xslabs.append(xs)

        def ctx_dma(mc):
            cs = cn_pool.tile([P, CC, 512], BF16, tag="cs")
            nc.sync.dma_start(
                cs[:], cT_d[:, mc * 512:(mc + 1) * 512]
                .rearrange("(cc p) n -> p cc n", p=P))
            cslabs[mc] = cs

        ctx_dma(0)
        ctx_dma(1)
        x_dma(0)
        ctx_dma(2)
        ctx_dma(3)
        x_dma(1)

        bar_in = dram.tile([1, 4], F32, tag="barin")
        bar_out = dram.tile([R, 4], F32, tag="barout")
        nc.scalar.dma_start(bar_in[:], bq_d.rearrange("d o -> o d")[0:1, 0:4])
        nc.gpsimd.collective_compute(
            "AllGather", OP.bypass, replica_groups=[list(range(R))],
            ins=[bar_in.opt()], outs=[bar_out.opt()])

        def q_proj(b):
            pq = psp.tile([P, 512], F32, tag="p")
            for cc in range(CC):
                nc.tensor.matmul(pq[:], lhsT=wq_t[:, cc, :], rhs=xslabs[b][:, cc, :],
                                 start=(cc == 0), stop=(cc == CC - 1))
            qt = qt_pool.tile([P, 512], BF16, tag="qt")
            nc.scalar.activation(qt[:], pq[:], AF.Identity, bias=bq_t[:], scale=1.0)
            qt_tiles.append(qt)

        # ---- Phase B slab chunks (emitted inline or as attention filler) ----
        def slab_chunks(mc):
            def c_k():
                pk = psp.tile([P, 512], F32, tag="p", name=f"pk{mc}")
                for cc in range(CC):
                    nc.tensor.matmul(pk[:], lhsT=wk_t[:, cc, :],
                                     rhs=cslabs[mc][:, cc, :],
                                     start=(cc == 0), stop=(cc == CC - 1))
                kt = kt_pool.tile([P, 512], BF16, tag="kt")
                nc.scalar.activation(kt[:], pk[:], AF.Identity, bias=bk_t[:], scale=1.0)
                kt_tiles.append(kt)

            def c_v():
                pv = psp.tile([P, 512], F32, tag="p", name=f"pv{mc}")
                for cc in range(CC):
                    nc.tensor.matmul(pv[:], lhsT=wv_t[:, cc, :],
                                     rhs=cslabs[mc][:, cc, :],
                                     start=(cc == 0), stop=(cc == CC - 1))
                vt_sb = vt_pool.tile([P, 512], F32R, tag="vt")
                nc.vector.tensor_copy(vt_sb[:], pv[:])
                cslabs[mc] = None
                return vt_sb

            def c_t(vt_sb):
                pvt = pst.tile([P, 512], F32R, tag="t")
                for j in range(4):
                    nc.tensor.transpose(pvt[:, j * P:(j + 1) * P],
                                        vt_sb[:, j * P:(j + 1) * P], identr[:])
                vn = vn_pool.tile([P, 4, 2, 65], BF16, tag="vn")
                nc.vector.tensor_copy(
                    vn[:, :, :, 0:64],
                    pvt[:].rearrange("p (j a d) -> p j a d", j=4, a=2))
                nc.vector.tensor_copy(vn[:, :, :, 64:65],
                                      mcolt[:, mc * 4:(mc + 1) * 4, :])
                vn_tiles.append(vn)

            state = {}
            def chunk1(): state['v'] = None; c_k()
            def chunk2(): state['v'] = c_v()
            def chunk3(): c_t(state['v'])
            return [chunk1, chunk2, chunk3]

        # ---- attention per batch, with filler chunks interleaved ----
        def attention_batch(b, filler):
            pav = [psa.tile([P, 512], F32, tag="a", name=f"pav{_h}") for _h in range(2)]
            fi = 0
            for mt in range(MT):
                tm = b * MT + mt
                mc, j = tm // 4, tm % 4
                kt = kt_tiles[mc]
                vn = vn_tiles[mc]
                ps = [pss.tile([P, 512], F32, tag="s", name=f"ps{_h}") for _h in range(2)]
                for h in range(2):
                    nc.tensor.matmul(ps[h][:],
                                     lhsT=kt[h * 64:(h + 1) * 64, j * P:(j + 1) * P],
                                     rhs=qt_tiles[b][h * 64:(h + 1) * 64, :],
                                     start=True, stop=True)
                ee = []
                for h in range(2):
                    e = e_pool.tile([P, 512], BF16, tag="e")
                    nc.scalar.activation(e[:], ps[h][:], AF.Exp,
                                         bias=0.0, scale=float(SCALE))
                    ee.append(e)
                if fi < len(filler):
                    filler[fi]()
                    fi += 1
                first, last = (mt == 0), (mt == MT - 1)
                for h in range(2):
                    # lhsT = [V_h | mask]: rows 0:64 = (EV)^T, row 64 = denom
                    nc.tensor.matmul(pav[h][0:65, :], lhsT=vn[:, j, h, :],
                                     rhs=ee[h][:], start=first, stop=last)
            while fi < len(filler):
                filler[fi]()
                fi += 1
            avt = av_pool.tile([P, 512], BF16, tag="av")
            for h in range(2):
                den = nrm_pool.tile([1, 512], F32, tag="den")
                nc.vector.tensor_copy(den[:], pav[h][64:65, :])
                rec = nrm_pool.tile([1, 512], F32, tag="rec")
                nc.vector.reciprocal_approx_fast(rec[:], den[:])
                rec_b = nrm_pool.tile([1, 512], BF16, tag="recb")
                nc.vector.tensor_copy(rec_b[:], rec[:])
                pb = psp.tile([P, 512], F32, tag="p")
                nc.tensor.matmul(pb[0:64, :], lhsT=ones_b[:], rhs=rec_b[:],
                                 start=True, stop=True)
                bc = nrm_pool.tile([64, 512], F32, tag="bc")
                nc.vector.tensor_copy(bc[:], pb[0:64, :])
                nc.vector.scalar_tensor_tensor(
                    out=avt[h * 64:(h + 1) * 64, :], in0=pav[h][0:64, :],
                    scalar=1.0, in1=bc[:], op0=OP.mult, op1=OP.mult)
            av_tiles.append(avt)
            hh = b // 2
            if b % 2 == 0:
                a2a_in = dram.tile([R, P, P], BF16, name=f"a2ai{hh}")
                a2a_ins.append(a2a_in)
            else:
                a2a_in = a2a_ins[hh]
            # stage this batch's 4 chunks now (slot j gets batch 2hh+j//4)
            for j in range(R):
                if j // 4 != b % 2:
                    continue
                nc.scalar.dma_start(a2a_in[j, :, :],
                                    avt[:, (j % 4) * P:(j % 4 + 1) * P])
            if b % 2 == 1:
                a2a_out = dram.tile([R, P, P], BF16, name=f"a2ao{hh}")
                nc.gpsimd.collective_compute(
                    "AllToAll", OP.bypass, replica_groups=[list(range(R))],
                    ins=[a2a_in.opt()], outs=[a2a_out.opt()])
                a2a_outs.append(a2a_out)

        def wo_chunks(h):
            """Deferred behind the a2a so the collective completes off the
            critical path (in-order engine queues)."""
            state = {}

            def load():
                agt = agp.tile([P, CC, P], BF16, name=f"agt{h}", tag="ag")
                nc.scalar.dma_start(agt[:], a2a_outs[h].rearrange("i p n -> p i n"))
                state['agt'] = agt

            def ch_chunk(ch):
                agt = state['agt']
                po = psp.tile([P, 512], F32, tag="p", name=f"po{h}_{ch}")
                for cc in range(CC):
                    nc.tensor.matmul(po[:], lhsT=agt[:, cc, :],
                                     rhs=wo_t[:, cc, ch * 512:(ch + 1) * 512],
                                     start=(cc == 0), stop=(cc == CC - 1))
                ob = outp.tile([P, 512], BF16, tag="ob", name=f"ob{h}_{ch}")
                nc.vector.scalar_tensor_tensor(
                    out=ob[:], in0=po[:], scalar=1.0,
                    in1=bo_t[:, ch * 512:(ch + 1) * 512],
                    op0=OP.mult, op1=OP.add)
                nc.scalar.dma_start(out_d[h, :, ch * 512:(ch + 1) * 512], ob[:])

            return [load, lambda: ch_chunk(0), lambda: ch_chunk(1)]

        # slabs 0-3 inline (batch 0's keys), then attention batches with the
        # next batch's slab chunks (or wo_half(0)) as PE filler.
        # prologue: slabs 0-3 + q-projections, ctx dmas rolling 4 ahead
        chunks0 = slab_chunks(0)
        chunks1 = slab_chunks(1)
        chunks0[0](); chunks0[1](); chunks0[2]()
        ctx_dma(4)
        q_proj(0)
        chunks1[0](); chunks1[1](); chunks1[2]()
        ctx_dma(5)
        q_proj(1)
        for c in slab_chunks(2):
            c()
        x_dma(2)
        ctx_dma(6)
        q_proj(2)
        for c in slab_chunks(3):
            c()
        x_dma(3)
        ctx_dma(7)
        q_proj(3)
        wo_dmaed = False
        for b in range(B):
            if b < 3:
                filler = []
                for k in range(4):
                    mc = 4 * (b + 1) + k
                    cks = slab_chunks(mc)
                    def mk_tail(ck3, nxt):
                        def f():
                            ck3()
                            if nxt is not None:
                                ctx_dma(nxt)
                        return f
                    nxt = mc + 4 if mc + 4 < 16 else None
                    filler += [cks[0], cks[1], mk_tail(cks[2], nxt)]
            else:
                noop = lambda: None
                w0 = wo_chunks(0)
                filler = [noop] * 6 + [w0[0], noop, w0[1], noop, w0[2]]
            attention_batch(b, filler)
            if b == 1 and not wo_dmaed:
                # wo arrives behind all ctx slabs, ahead of first use
                nc.sync.dma_start(wo_t[:], wo_d.rearrange("(cc p) c -> p cc c", p=P))
                wo_dmaed = True
        for c in wo_chunks(1):
            c()

        es2.close()

    nc.compile()
    return nc


def _get_nc():
    if "nc" not in _cached:
        _cached["nc"] = _build()
    return _cached["nc"]


def _prep_inputs(x, context, ctx_key_padding_mask, Wq, bq, Wk, bk, Wv, bv, Wo, bo):
    bf16 = ml_dtypes.bfloat16
    x = np.asarray(x, dtype=np.float32).reshape(BN, C)
    ctx = np.asarray(context, dtype=np.float32).reshape(BM, C)
    mask = np.asarray(ctx_key_padding_mask, dtype=np.int32)
    Wq = np.asarray(Wq, dtype=np.float32)
    Wk = np.asarray(Wk, dtype=np.float32)
    Wv = np.asarray(Wv, dtype=np.float32)
    Wo = np.asarray(Wo, dtype=np.float32)
    bq = np.asarray(bq, dtype=np.float32)
    bk = np.asarray(bk, dtype=np.float32)
    bv = np.asarray(bv, dtype=np.float32)
    bo = np.asarray(bo, dtype=np.float32)
    # bv folds through the (row-stochastic) attention and Wo exactly:
    # out = (attn + bv) @ Wo.T + bo = attn @ Wo.T + (bo + Wo @ bv)
    bo_eff = (bo.astype(np.float64) + Wo.astype(np.float64) @ bv.astype(np.float64)
              ).astype(np.float32)
    bo_bc = np.ascontiguousarray(np.broadcast_to(bo_eff, (P, C)))
    # key-padding mask folded into ctx^T: masked keys' columns become 0, so
    # v is masked for free; their k rows are bk-only but those E values are
    # multiplied by the zero mask column in both softmax sums.
    mf = (mask != 0).astype(np.float32).reshape(BM)      # [BM]
    xT = np.ascontiguousarray(x.T).astype(bf16)          # [C, BN]
    cT = np.ascontiguousarray((ctx * mf[:, None]).T).astype(bf16)  # [C, BM]
    # mcol[p, tm, h] = mask value for key tm*128+p (same for both heads)
    mcol = np.ascontiguousarray(
        np.broadcast_to(mf.reshape(TM, P).T[:, :, None], (P, TM, 2))).astype(bf16)
    wo_full = np.ascontiguousarray(Wo.T).astype(bf16)
    in_maps = []
    for r in range(R):
        sl = slice(r * DL, (r + 1) * DL)
        in_maps.append({
            "xT": xT, "cT": cT, "mcol": mcol,
            "wq": np.ascontiguousarray(Wq[sl, :].T).astype(bf16),
            "wk": np.ascontiguousarray(Wk[sl, :].T).astype(bf16),
            "wv": np.ascontiguousarray(Wv[sl, :].T).astype(bf16),
            "wo": wo_full,
            "bq": np.ascontiguousarray(bq[sl].reshape(DL, 1)),
            "bk": np.ascontiguousarray(bk[sl].reshape(DL, 1)),
            "bo": bo_bc,
        })
    return in_maps


def _run(in_maps, **kwargs):
    from concourse.bass_utils import run_bass_kernel_spmd
    nc = _get_nc()
    return run_bass_kernel_spmd(nc, in_maps, list(range(R)), **kwargs)


def kernel(x, context, ctx_key_padding_mask, Wq, bq, Wk, bk, Wv, bv, Wo, bo):
    in_maps = _prep_inputs(x, context, ctx_key_padding_mask,
                           Wq, bq, Wk, bk, Wv, bv, Wo, bo)
    res = _run(in_maps).results
    out = np.empty((BN, C), dtype=np.float32)
    for r in range(R):
        o = res[r]["out"]          # [2, 128, C]: half h -> batch 2h + r//4,
        for h in range(2):         # rows (r%4)*128 ...
            b = 2 * h + r // 4
            row = b * N + (r % 4) * P
            out[row:row + P] = o[h]
    return np.ascontiguousarray(out.reshape(B, N, C))
